# revision 1
# baseline (speedup 1.0000x reference)
# Trainium2 Bass kernel for nn_Edge_CNN (DynamicEdgeConv x3 + PairNorm + pool + MLP head).
#
# Strategy: data-parallel over the 32 graphs -> 8 NeuronCores x 4 graphs. PairNorm
# couples all graphs (stats over the whole batch), handled with a tiny per-layer
# AllReduce of per-channel sums + total square sum.
#
# Per-core program (all feature tensors channel-major [C<=128 part, N=1024 free]):
#   EdgeConv factorization: msg_ij = u_i + v_j, u = x@(Wa-Wb)+b, v = x@Wb; the
#   per-edge MLP collapses into two node transforms + a kNN-masked max of v.
#   kNN selection on negdist_ij = 2*x_i.x_j - |x_j|^2 (row-constant shifts don't
#   change per-row top-k): PE computes it straight into PSUM via an augmented
#   (ones x -|x_j|^2) rank-1 matmul; DVE max8/max_index/match_replace extract the
#   exact top-10 (+6 duplicate pad slots). Aggregation: gpsimd ap_gather
#   gathers v at the 16 slot indices (shared across channel partitions), DVE
#   segmented reduce-max collapses the slots, then h = relu(u + maxv).
#
# kernel(**inputs) takes the FULL unsharded inputs and returns the FULL [32, 2].

import numpy as np
from contextlib import ExitStack

import concourse.bass as bass
import concourse.bacc as bacc
import concourse.mybir as mybir
import concourse.tile as tile
from concourse.bass_utils import run_bass_kernel_spmd

N = 1024
B_TOTAL = 32
N_CORES = 8
G = B_TOTAL // N_CORES
F32 = mybir.dt.float32
U16 = mybir.dt.uint16
I16 = mybir.dt.int16
AF = mybir.ActivationFunctionType
ALU = mybir.AluOpType
AX = mybir.AxisListType
NCHUNK = N // 128
LAYERS = [(2, 64), (64, 128), (128, 256)]  # (D_in, C_out)


def _ccdiv(c):
    return (c + 127) // 128


def _build(tc, nc, ins, outs, n_cores, eps=1e-5):
    TOTAL_NODES = float(B_TOTAL * N)
    replica = [list(range(n_cores))]

    es = ExitStack()
    sb = es.enter_context(tc.tile_pool(name="sb", bufs=1))
    xp = es.enter_context(tc.tile_pool(name="xp", bufs=12))
    wk = es.enter_context(tc.tile_pool(name="wk", bufs=3))
    sm = es.enter_context(tc.tile_pool(name="sm", bufs=4))
    gt = es.enter_context(tc.tile_pool(name="gt", bufs=2))
    ps_g = es.enter_context(tc.tile_pool(name="ps_g", bufs=2, space="PSUM"))
    ps_v = es.enter_context(tc.tile_pool(name="ps_v", bufs=2, space="PSUM"))
    dr = es.enter_context(tc.tile_pool(name="dr", bufs=1, space="DRAM"))

    ones1 = sb.tile([1, 128], F32, tag="ones1")
    nc.vector.memset(ones1[:], 1.0)
    ones128 = sb.tile([128, 1], F32, tag="ones128")
    nc.vector.memset(ones128[:], 1.0)

    W = {}
    for li, (D, C) in enumerate(LAYERS):
        for nm in ("wab", "wb"):
            t = sb.tile([D, C], F32, tag=f"{nm}{li}", name=f"{nm}{li}")
            nc.sync.dma_start(t[:], ins[f"{nm}{li}"][:])
            W[f"{nm}{li}"] = t
        t = sb.tile([min(C, 128), _ccdiv(C)], F32, tag=f"b{li}", name=f"b{li}")
        nc.sync.dma_start(t[:], ins[f"b{li}"][:].rearrange("(cc p) one -> p (cc one)",
                                                           p=min(C, 128)))
        W[f"b{li}"] = t
    wl1 = sb.tile([128, 2, 64], F32, tag="wl1")
    nc.sync.dma_start(wl1[:], ins["wl1"][:].rearrange("cc p c -> p cc c"))
    wl2 = sb.tile([64, 2], F32, tag="wl2")
    nc.sync.dma_start(wl2[:], ins["wl2"][:])
    bl1 = sb.tile([64, 1], F32, tag="bl1")
    nc.sync.dma_start(bl1[:], ins["bl1"][:])
    bl2 = sb.tile([2, 1], F32, tag="bl2")
    nc.sync.dma_start(bl2[:], ins["bl2"][:])

    xT = {}
    for g in range(G):
        t = xp.tile([128, N], F32, tag="xh", name=f"x0_{g}")
        nc.sync.dma_start(t[0:2, :], ins["pos"][g, :, :].rearrange("j d -> d j"))
        xT[(g, 0)] = t

    def unit(li, D, C, g):
        CC = _ccdiv(C)
        xt = xT[(g, 0)]

        rhs2x = wk.tile([128, N], F32, tag="rhs2x")
        nc.vector.tensor_scalar_mul(rhs2x[0:D, :], xt[0:D, :], 2.0)
        sq = wk.tile([128, N], F32, tag="sqx")
        nc.vector.tensor_tensor(sq[0:D, :], xt[0:D, :], xt[0:D, :], ALU.mult)
        psq = ps_v.tile([1, N], F32, tag="pvu")
        onesD = sm.tile([128, 1], F32, tag="onesD")
        nc.vector.memset(onesD[0:D, :], 1.0)
        for b in range(2):
            nc.tensor.matmul(psq[:, 512 * b:512 * (b + 1)], onesD[0:D, :],
                             sq[0:D, 512 * b:512 * (b + 1)], start=True, stop=True)
        rhsq = wk.tile([1, N], F32, tag="rhsq")
        nc.scalar.activation(rhsq[:], psq[:], AF.Copy, scale=-1.0)

        wrapIdx = sm.tile([32, N], I16, tag="wrapIdx")
        idx16 = sm.tile([128, 32], I16, tag="idx16")
        nc.vector.memset(idx16[:], 0)

        for c in range(NCHUNK):
            pg = ps_g.tile([128, N], F32, tag="gram")
            for b in range(2):
                sl = slice(512 * b, 512 * (b + 1))
                nc.tensor.matmul(pg[:, sl], xt[0:D, 128 * c:128 * (c + 1)],
                                 rhs2x[0:D, sl], start=True, stop=False)
                nc.tensor.matmul(pg[:, sl], ones1[:], rhsq[:, sl], start=False, stop=True)
            nd = wk.tile([128, N], F32, tag="negdist")
            nc.scalar.activation(nd[:], pg[:], AF.Copy)
            mx1 = sm.tile([128, 8], F32, tag="mx1")
            mi1 = sm.tile([128, 8], U16, tag="mi1")
            mx2 = sm.tile([128, 8], F32, tag="mx2")
            mi2 = sm.tile([128, 8], U16, tag="mi2")
            nc.vector.max(mx1[:], nd[:])
            nc.vector.max_index(mi1[:], mx1[:], nd[:])
            nc.vector.match_replace(nd[:], mx1[:], nd[:], -1e30)
            nc.vector.max(mx2[:], nd[:])
            nc.vector.max_index(mi2[:], mx2[:], nd[:])
            nc.vector.tensor_copy(idx16[:, 0:8], mi1[:])
            nc.vector.tensor_copy(idx16[:, 8:10], mi2[:, 0:2])
            nc.vector.tensor_copy(idx16[:, 10:16], mi1[:, 0:6])
            for a in range(4):
                nc.vector.transpose(wrapIdx[0:32, 128 * c + 32 * a:128 * c + 32 * (a + 1)],
                                    idx16[32 * a:32 * (a + 1), 0:32])

        repIdx = sm.tile([128, N], I16, tag="repIdx")
        for grp in range(8):
            nc.sync.dma_start(repIdx[16 * grp:16 * (grp + 1), :], wrapIdx[0:16, :])

        hs = {}
        for cc in range(CC):
            cw = min(128, C - 128 * cc)
            csl = slice(128 * cc, 128 * cc + cw)
            pv = ps_v.tile([128, N], F32, tag="pvu")
            for b in range(2):
                sl = slice(512 * b, 512 * (b + 1))
                nc.tensor.matmul(pv[0:cw, sl], W[f"wb{li}"][0:D, csl], xt[0:D, sl],
                                 start=True, stop=True)
            vT = sm.tile([128, N], F32, tag="vT")
            nc.scalar.activation(vT[0:cw, :], pv[0:cw, :], AF.Copy)
            if cw < 128:
                nc.vector.memset(vT[cw:128, :], 0)
            pu = ps_v.tile([128, N], F32, tag="pvu")
            for b in range(2):
                sl = slice(512 * b, 512 * (b + 1))
                nc.tensor.matmul(pu[0:cw, sl], W[f"wab{li}"][0:D, csl], xt[0:D, sl],
                                 start=True, stop=True)
            uT = wk.tile([128, N], F32, tag="uT")
            nc.scalar.activation(uT[0:cw, :], pu[0:cw, :], AF.Identity,
                                 bias=W[f"b{li}"][0:cw, cc:cc + 1])
            h = xp.tile([128, N], F32, tag="xh", name=f"h{li}_{g}_{cc}")
            for part in range(4):
                hn = N // 4
                hsl = slice(part * hn, (part + 1) * hn)
                gout = gt.tile([128, 16 * hn], F32, tag="gout", name=f"gout{part}")
                nc.gpsimd.ap_gather(gout[:], vT[:], repIdx[:, hsl],
                                    channels=128, num_elems=N, d=1, num_idxs=16 * hn)
                nc.vector.tensor_reduce(h[0:cw, hsl],
                                        gout[0:cw, :].rearrange("p (i s) -> p i s", s=16),
                                        AX.X, ALU.max)
            nc.vector.tensor_tensor(h[0:cw, :], h[0:cw, :], uT[0:cw, :], ALU.add)
            rsum = sm.tile([128, 1], F32, tag="rsum")
            nc.scalar.activation(h[0:cw, :], h[0:cw, :], AF.Relu, accum_out=rsum[0:cw, :])
            sqh = wk.tile([128, N], F32, tag="sqh")
            qsum = sm.tile([128, 1], F32, tag="qsum")
            nc.scalar.activation(sqh[0:cw, :], h[0:cw, :], AF.Square, accum_out=qsum[0:cw, :])
            hs[cc] = (h, rsum, qsum, cw)
        return hs

    for li, (D, C) in enumerate(LAYERS):
        CC = _ccdiv(C)
        ssum = [sm.tile([128, G], F32, tag=f"ssum{li}_{cc}", name=f"ssum{li}_{cc}")
                for cc in range(CC)]
        qsum = [sm.tile([128, G], F32, tag=f"qsum{li}_{cc}", name=f"qsums{li}_{cc}")
                for cc in range(CC)]
        hsg = {}
        for g in range(G):
            hs = unit(li, D, C, g)
            for cc, (h, rs, qs, cw) in hs.items():
                nc.vector.tensor_copy(ssum[cc][0:cw, g:g + 1], rs[0:cw, :])
                nc.vector.tensor_copy(qsum[cc][0:cw, g:g + 1], qs[0:cw, :])
                hsg[(g, cc)] = (h, cw)
        stat = sm.tile([128, 2 * CC], F32, tag=f"stat{li}", name=f"stat{li}")
        nc.vector.memset(stat[:], 0.0)
        for cc in range(CC):
            cw = min(128, C - 128 * cc)
            nc.vector.tensor_reduce(stat[0:cw, 2 * cc:2 * cc + 1], ssum[cc][0:cw, :],
                                    AX.X, ALU.add)
            nc.vector.tensor_reduce(stat[0:cw, 2 * cc + 1:2 * cc + 2], qsum[cc][0:cw, :],
                                    AX.X, ALU.add)
        bi = dr.tile([128, 2 * CC], F32, tag=f"cc_in{li}", name=f"cc_in{li}")
        bo = dr.tile([128, 2 * CC], F32, tag=f"cc_out{li}", name=f"cc_out{li}")
        nc.gpsimd.dma_start(bi[:], stat[:])
        nc.gpsimd.collective_compute("AllReduce", ALU.add, replica_groups=replica,
                                     ins=[bi[:].opt()], outs=[bo[:].opt()])
        statg = sm.tile([128, 2 * CC], F32, tag=f"statg{li}", name=f"statg{li}")
        nc.gpsimd.dma_start(statg[:], bo[:])
        mu = sm.tile([128, CC], F32, tag=f"mu{li}", name=f"mu{li}")
        musq = sm.tile([128, CC], F32, tag=f"musq{li}", name=f"musq{li}")
        for cc in range(CC):
            nc.vector.tensor_scalar_mul(mu[:, cc:cc + 1], statg[:, 2 * cc:2 * cc + 1],
                                        1.0 / TOTAL_NODES)
        nc.vector.tensor_tensor(musq[:], mu[:], mu[:], ALU.mult)
        pr = ps_v.tile([1, 2], F32, tag="pvu")
        for cc in range(CC):
            nc.tensor.matmul(pr[:, 0:1], statg[:, 2 * cc + 1:2 * cc + 2], ones128[:],
                             start=(cc == 0), stop=(cc == CC - 1))
        for cc in range(CC):
            nc.tensor.matmul(pr[:, 1:2], musq[:, cc:cc + 1], ones128[:],
                             start=(cc == 0), stop=(cc == CC - 1))
        sc = sm.tile([1, 2], F32, tag=f"sc{li}", name=f"sc{li}")
        nc.scalar.activation(sc[:], pr[:], AF.Copy)
        rsc = sm.tile([1, 1], F32, tag=f"rsc{li}", name=f"rsc{li}")
        nc.vector.tensor_scalar(rsc[:], sc[:, 0:1], scalar1=1.0 / TOTAL_NODES, scalar2=eps,
                                op0=ALU.mult, op1=ALU.add)
        nc.vector.tensor_tensor(rsc[:], rsc[:], sc[:, 1:2], ALU.subtract)
        nc.vector.reciprocal(rsc[:], rsc[:])
        nc.scalar.activation(rsc[:], rsc[:], AF.Sqrt)
        prb = ps_v.tile([128, 1], F32, tag="pvu")
        nc.tensor.matmul(prb[:], ones1[:], rsc[:], start=True, stop=True)
        rcol = sm.tile([128, 1], F32, tag=f"rcol{li}", name=f"rcol{li}")
        nc.scalar.activation(rcol[:], prb[:], AF.Copy)
        muR = sm.tile([128, CC], F32, tag=f"muR{li}", name=f"muR{li}")
        nc.vector.tensor_scalar(muR[:], mu[:], scalar1=rcol[:], scalar2=None, op0=ALU.mult)
        for g in range(G):
            for cc in range(CC):
                h, cw = hsg[(g, cc)]
                nc.vector.tensor_scalar(h[0:cw, :], h[0:cw, :], scalar1=rcol[0:cw, :],
                                        scalar2=muR[0:cw, cc:cc + 1],
                                        op0=ALU.mult, op1=ALU.subtract)
                xT[(g, cc)] = h

    gmat = sm.tile([128, 2, G], F32, tag="gmat")
    for g in range(G):
        for cc in range(2):
            nc.vector.tensor_reduce(gmat[:, cc, g:g + 1], xT[(g, cc)][:], AX.X, ALU.max)
    ph = ps_v.tile([64, G], F32, tag="pvu")
    for cc in range(2):
        nc.tensor.matmul(ph[:], wl1[:, cc, :], gmat[:, cc, :], start=(cc == 0), stop=(cc == 1))
    hh = sm.tile([64, G], F32, tag="hh")
    nc.scalar.activation(hh[:], ph[:], AF.Relu, bias=bl1[:])
    po = ps_v.tile([2, G], F32, tag="pvu")
    nc.tensor.matmul(po[:], wl2[:], hh[:], start=True, stop=True)
    oo = sm.tile([2, G], F32, tag="oo")
    nc.scalar.activation(oo[:], po[:], AF.Identity, bias=bl2[:])
    nc.sync.dma_start(outs["out"][:], oo[:])
    es.close()


def _host_weights(inputs):
    w = {}
    for li, (D, C) in enumerate(LAYERS):
        Wl = np.asarray(inputs[f"W{li + 1}"], dtype=np.float32)
        w[f"wab{li}"] = np.ascontiguousarray(Wl[:D] - Wl[D:])
        w[f"wb{li}"] = np.ascontiguousarray(Wl[D:])
        w[f"b{li}"] = np.ascontiguousarray(
            np.asarray(inputs[f"b{li + 1}"], dtype=np.float32).reshape(C, 1))
    w["wl1"] = np.ascontiguousarray(
        np.asarray(inputs["Wl1"], dtype=np.float32).reshape(2, 128, 64))
    w["bl1"] = np.asarray(inputs["bl1"], dtype=np.float32).reshape(64, 1).copy()
    w["wl2"] = np.ascontiguousarray(np.asarray(inputs["Wl2"], dtype=np.float32))
    w["bl2"] = np.asarray(inputs["bl2"], dtype=np.float32).reshape(2, 1).copy()
    return w


_CACHED = {}


def _get_module():
    if "nc" in _CACHED:
        return _CACHED["nc"]
    nc = bacc.Bacc("TRN2", target_bir_lowering=False, debug=False, num_devices=N_CORES)
    ins = {"pos": nc.dram_tensor("pos", (G, N, 2), F32, kind="ExternalInput")}
    for li, (D, C) in enumerate(LAYERS):
        ins[f"wab{li}"] = nc.dram_tensor(f"wab{li}", (D, C), F32, kind="ExternalInput")
        ins[f"wb{li}"] = nc.dram_tensor(f"wb{li}", (D, C), F32, kind="ExternalInput")
        ins[f"b{li}"] = nc.dram_tensor(f"b{li}", (C, 1), F32, kind="ExternalInput")
    ins["wl1"] = nc.dram_tensor("wl1", (2, 128, 64), F32, kind="ExternalInput")
    ins["bl1"] = nc.dram_tensor("bl1", (64, 1), F32, kind="ExternalInput")
    ins["wl2"] = nc.dram_tensor("wl2", (64, 2), F32, kind="ExternalInput")
    ins["bl2"] = nc.dram_tensor("bl2", (2, 1), F32, kind="ExternalInput")
    outs = {"out": nc.dram_tensor("out", (2, G), F32, kind="ExternalOutput")}
    with tile.TileContext(nc) as tc:
        _build(tc, nc, ins, outs, n_cores=N_CORES)
    nc.compile()
    _CACHED["nc"] = nc
    return nc


def kernel(**inputs):
    pos = np.ascontiguousarray(np.asarray(inputs["pos"], dtype=np.float32))
    w = _host_weights(inputs)
    nc = _get_module()
    in_maps = []
    for core in range(N_CORES):
        m = {"pos": np.ascontiguousarray(pos[core * G:(core + 1) * G])}
        m.update(w)
        in_maps.append(m)
    res = run_bass_kernel_spmd(nc, in_maps, list(range(N_CORES)))
    outs = [res.results[c]["out"].T for c in range(N_CORES)]  # each [G, 2]
    return np.concatenate(outs, axis=0).astype(np.float32)



# revision 10
# speedup vs baseline: 1.2219x; 1.2219x over previous
# Trainium2 Bass kernel for nn_Edge_CNN (DynamicEdgeConv x3 + PairNorm + pool + MLP head).
#
# Data-parallel over the 32 graphs -> 8 NeuronCores x 4 graphs. PairNorm couples
# graphs only through per-channel mean + a scalar; those stats go through a tiny
# AllReduce whose result is folded into the NEXT layer's activation (scale r,
# bias b - r*Wa^T mu), so all heavy per-layer compute runs on unnormalized
# activations (kNN selection is invariant under the shared affine transform).
#
# Top-k selection per 128-row chunk packs the candidate index into the low 10
# mantissa bits of the (quantized) negative-distance fp32 ("keys"), so a single
# max8 pass yields values AND indices: stt-keys -> max8 -> match_replace ->
# max8 gives ranks 1..16; the kNN gather streams 16 slots/node but the slot-max
# reduce reads only slots 0..10 (k=10 exact).
#
# kernel(**inputs) takes FULL unsharded inputs, returns the FULL [32, 2].

import numpy as np
from contextlib import ExitStack

import concourse.bass as bass
import concourse.bacc as bacc
import concourse.mybir as mybir
import concourse.tile as tile
from concourse.bass_utils import run_bass_kernel_spmd

N = 1024
B_TOTAL = 32
N_CORES = 8
G = B_TOTAL // N_CORES
F32 = mybir.dt.float32
U32 = mybir.dt.uint32
U16 = mybir.dt.uint16
I16 = mybir.dt.int16
AF = mybir.ActivationFunctionType
ALU = mybir.AluOpType
AX = mybir.AxisListType
NCHUNK = N // 128
LAYERS = [(2, 64), (64, 128), (128, 256)]  # (D_in, C_out)
KSLOT = 16   # gather slots per node (ranks 1..16; reduce uses first 10)
KTOP = 10
NQ = 4       # gather split: quarters per (unit, cc)
HQ = N // NQ


def _ccdiv(c):
    return (c + 127) // 128


def _build(tc, nc, ins, outs, n_cores, eps=1e-5):
    TOTAL_NODES = float(B_TOTAL * N)
    replica = [list(range(n_cores))]

    es = ExitStack()
    sb = es.enter_context(tc.tile_pool(name="sb", bufs=1))
    feat = es.enter_context(tc.tile_pool(name="feat", bufs=14))
    kp = es.enter_context(tc.tile_pool(name="kp", bufs=2))
    wk = es.enter_context(tc.tile_pool(name="wk", bufs=2))
    sm = es.enter_context(tc.tile_pool(name="sm", bufs=4))
    gt = es.enter_context(tc.tile_pool(name="gt", bufs=2))
    ps_g = es.enter_context(tc.tile_pool(name="ps_g", bufs=2, space="PSUM"))
    ps_v = es.enter_context(tc.tile_pool(name="ps_v", bufs=2, space="PSUM"))
    dr = es.enter_context(tc.tile_pool(name="dr", bufs=1, space="DRAM"))

    # ---- one-time constants ----
    ones1 = sb.tile([1, 128], F32, tag="ones1")
    nc.vector.memset(ones1[:], 1.0)
    ones128 = sb.tile([128, 1], F32, tag="ones128")
    nc.vector.memset(ones128[:], 1.0)
    iota = sb.tile([128, N], U32, tag="iota")
    nc.gpsimd.iota(iota[:], pattern=[[1, N]], base=0, channel_multiplier=0)
    kmask = sb.tile([128, 1], U32, tag="kmask")
    nc.vector.memset(kmask[:], 0xFFFFFC00)
    kmask10 = sb.tile([128, 1], U16, tag="kmask10")
    nc.vector.memset(kmask10[:], 0x3FF)
    # persistent top-16 key tiles (pad cols 16:32 zeroed once; max8 writes 0:16)
    ktops = []
    for i in range(2):
        t = sb.tile([128, 32], F32, tag=f"ktop{i}")
        nc.vector.memset(t[:, 16:32], 0.0)
        ktops.append(t)

    # ---- weights ----
    W = {}
    for li, (D, C) in enumerate(LAYERS):
        DCC = _ccdiv(D)
        for nm in ("wab", "wb", "wa"):
            for dc in range(DCC):
                dw = min(128, D - 128 * dc)
                t = sb.tile([dw, C], F32, tag=f"{nm}{li}_{dc}", name=f"{nm}{li}_{dc}")
                nc.sync.dma_start(t[:], ins[f"{nm}{li}"][128 * dc:128 * dc + dw, :])
                W[(nm, li, dc)] = t
        cw0 = min(C, 128)
        t = sb.tile([cw0, _ccdiv(C)], F32, tag=f"b{li}", name=f"b{li}")
        nc.sync.dma_start(t[:], ins[f"b{li}"][:].rearrange("(cc p) one -> p (cc one)",
                                                           p=cw0))
        W[("b", li)] = t
    wl1 = sb.tile([128, 2, 64], F32, tag="wl1")
    nc.sync.dma_start(wl1[:], ins["wl1"][:].rearrange("cc p c -> p cc c"))
    wl2 = sb.tile([64, 2], F32, tag="wl2")
    nc.sync.dma_start(wl2[:], ins["wl2"][:])
    bl1 = sb.tile([64, 1], F32, tag="bl1")
    nc.sync.dma_start(bl1[:], ins["bl1"][:])
    bl2 = sb.tile([2, 1], F32, tag="bl2")
    nc.sync.dma_start(bl2[:], ins["bl2"][:])

    # layer-0 packed bias [b0; b0] and scale r=1
    b0p = sb.tile([128, 1], F32, tag="b0p")
    nc.vector.tensor_copy(b0p[0:64, :], W[("b", 0)][:, 0:1])
    nc.vector.tensor_copy(b0p[64:128, :], W[("b", 0)][:, 0:1])
    ones_r = sb.tile([128, 1], F32, tag="ones_r")
    nc.vector.memset(ones_r[:], 1.0)

    # ---- load pos ----
    a_cur = {}  # (g, cc) -> (tile, rowlo, rowhi) feature rows of graph g chunk cc
    for g in range(G):
        t = feat.tile([128, N], F32, tag="feat", name=f"a0_{g}")
        nc.sync.dma_start(t[0:2, :], ins["pos"][g, :, :].rearrange("j d -> d j"))
        a_cur[(g, 0)] = (t, 0, 2)

    vector_or_pool = [nc.vector, nc.gpsimd]

    def topk_wrap(li, D, g, wrap, wrow):
        """Compute kNN top-16 index rows into wrap[wrow:wrow+32, :] for graph g.
        wrap rows wrow..wrow+16 become slots (ranks 1..16); rows +16..+32 garbage
        masked in-range. Caller replicates. D = feature dim of input."""
        DCC = _ccdiv(D)
        srcs = [a_cur[(g, dc)] for dc in range(DCC)]

        # column norms: psq[j] = sum_d a[d,j]^2 ; rhsq = -0.5*psq
        sq = wk.tile([128, N], F32, tag="sq")
        psq = ps_v.tile([1, N], F32, tag="pvu")
        for dc in range(DCC):
            t, lo, hi = srcs[dc]
            nc.scalar.activation(sq[0:hi - lo, :], t[lo:hi, :], AF.Square)
            for b in range(2):
                sl = slice(512 * b, 512 * (b + 1))
                nc.tensor.matmul(psq[:, sl], ones128[0:hi - lo, :], sq[0:hi - lo, sl],
                                 start=(dc == 0), stop=(dc == DCC - 1))
        rq = wk.tile([1, N], F32, tag="rq")
        nc.scalar.activation(rq[:], psq[:], AF.Copy, scale=-0.5)

        for c in range(NCHUNK):
            csl = slice(128 * c, 128 * (c + 1))
            pg = ps_g.tile([128, N], F32, tag="gram")
            for b in range(2):
                sl = slice(512 * b, 512 * (b + 1))
                for dc in range(DCC):
                    t, lo, hi = srcs[dc]
                    nc.tensor.matmul(pg[:, sl], t[lo:hi, csl], t[lo:hi, sl],
                                     start=(dc == 0), stop=False)
                nc.tensor.matmul(pg[:, sl], ones1[:], rq[:, sl], start=False, stop=True)
            keys = kp.tile([128, N], F32, tag="keys")
            nc.vector.scalar_tensor_tensor(keys[:].bitcast(U32), pg[:].bitcast(U32),
                                           kmask[:], iota[:],
                                           op0=ALU.bitwise_and, op1=ALU.bitwise_or)
            ktop = ktops[c % 2]
            nc.vector.max(ktop[:, 0:8], keys[:])
            nc.vector.match_replace(keys[:], ktop[:, 0:8], keys[:], -3.0e38)
            nc.vector.max(ktop[:, 8:16], keys[:])
            lo16 = ktop[:].bitcast(U16).rearrange("p (s two) -> p s two", two=2)
            for a in range(4):
                nc.vector.transpose(
                    wrap[wrow:wrow + 32, 128 * c + 32 * a:128 * c + 32 * (a + 1)]
                    .bitcast(U16),
                    lo16[32 * a:32 * (a + 1), :, 0])
        # mask to valid index range (also covers garbage rows wrow+16..wrow+32)
        nc.vector.tensor_scalar(wrap[wrow:wrow + 32, :].bitcast(U16),
                                wrap[wrow:wrow + 32, :].bitcast(U16),
                                scalar1=kmask10[wrow:wrow + 32, :], scalar2=None,
                                op0=ALU.bitwise_and)

    def gather_block(eng_i, wrap, vT, uT, h, cw):
        """h[0:cw,:] = max over slots 0..10 of gathered vT + uT (raw pre-act)."""
        for q in range(NQ):
            qsl = slice(HQ * q, HQ * (q + 1))
            gout = gt.tile([128, KSLOT * HQ], F32, tag="gout", name=f"gout{q}")
            nc.gpsimd.ap_gather(gout[:], vT[:], wrap[:, qsl],
                                channels=128, num_elems=N, d=1, num_idxs=KSLOT * HQ)
            nc.vector.tensor_reduce(h[0:cw, qsl],
                                    gout[0:cw, :].rearrange("p (i s) -> p i s",
                                                            s=KSLOT)[:, :, 0:KTOP],
                                    AX.X, ALU.max)
        nc.gpsimd.tensor_tensor(h[0:cw, :], h[0:cw, :], uT[0:cw, :], ALU.add)

    def uv_mats(li, D, g, csl, cw, vT, vrow, uT, urow):
        """vT[vrow:vrow+cw] = Wb^T a ; uT[urow:urow+cw] = Wab^T a (raw)."""
        DCC = _ccdiv(D)
        srcs = [a_cur[(g, dc)] for dc in range(DCC)]
        pv = ps_v.tile([128, N], F32, tag="pvu")
        for b in range(2):
            sl = slice(512 * b, 512 * (b + 1))
            for dc in range(DCC):
                t, lo, hi = srcs[dc]
                nc.tensor.matmul(pv[0:cw, sl], W[("wb", li, dc)][:, csl], t[lo:hi, sl],
                                 start=(dc == 0), stop=(dc == DCC - 1))
        nc.scalar.activation(vT[vrow:vrow + cw, :], pv[0:cw, :], AF.Copy)
        pu = ps_v.tile([128, N], F32, tag="pvu")
        for b in range(2):
            sl = slice(512 * b, 512 * (b + 1))
            for dc in range(DCC):
                t, lo, hi = srcs[dc]
                nc.tensor.matmul(pu[0:cw, sl], W[("wab", li, dc)][:, csl], t[lo:hi, sl],
                                 start=(dc == 0), stop=(dc == DCC - 1))
        nc.scalar.activation(uT[urow:urow + cw, :], pu[0:cw, :], AF.Copy)

    def relu_stats(a_t, arow, h, hrow, cw, rcol, badj_ap, rs, qs):
        """a = relu(h*r + badj); rs += rowsum(a), qs += rowsum(a^2) (written)."""
        nc.scalar.activation(a_t[arow:arow + cw, :], h[hrow:hrow + cw, :], AF.Relu,
                             scale=rcol, bias=badj_ap, accum_out=rs)
        sqh = wk.tile([128, N], F32, tag="sqh")
        nc.scalar.activation(sqh[0:cw, :], a_t[arow:arow + cw, :], AF.Square,
                             accum_out=qs)

    # ---- running normalization state ----
    rcol_prev = ones_r          # scale r_{l-1} as [128,1] AP
    badj = {0: b0p}             # layer0: bias = [b0;b0] packed (see L0 packing)

    stat_tiles = {}
    eng_rr = 0

    for li, (D, C) in enumerate(LAYERS):
        CC = _ccdiv(C)
        cw0 = min(C, 128)
        # stat accumulators: [128, 2*CC] per unit-slot collected below
        rsb = sm.tile([128, 2 * CC, G], F32, tag=f"rsb{li}", name=f"rsb{li}")

        if li == 0:
            # ---- layer 0: pack graph pairs (64ch each) into one gather ----
            for pair in range(2):
                gA, gB = 2 * pair, 2 * pair + 1
                wrap = kp.tile([128, N], I16, tag="wrap", name=f"wrap0_{pair}")
                topk_wrap(0, D, gA, wrap, 0)
                topk_wrap(0, D, gB, wrap, 64)
                # replicate: rows0:16 -> 16:32 via DMA; 0:32->32:64; gB at 64..96
                nc.sync.dma_start(wrap[16:32, :], wrap[0:16, :])
                nc.vector.tensor_copy(wrap[32:64, :], wrap[0:32, :])
                nc.sync.dma_start(wrap[80:96, :], wrap[64:80, :])
                nc.vector.tensor_copy(wrap[96:128, :], wrap[64:96, :])
                vT = wk.tile([128, N], F32, tag="vT", name=f"vT0_{pair}")
                uT = wk.tile([128, N], F32, tag="uT", name=f"uT0_{pair}")
                uv_mats(0, D, gA, slice(0, 64), 64, vT, 0, uT, 0)
                uv_mats(0, D, gB, slice(0, 64), 64, vT, 64, uT, 64)
                h = wk.tile([128, N], F32, tag="h", name=f"h0_{pair}")
                gather_block(eng_rr, wrap, vT, uT, h, 128)
                eng_rr += 1
                a_t = feat.tile([128, N], F32, tag="feat", name=f"a1_{pair}")
                relu_stats(a_t, 0, h, 0, 128, rcol_prev, badj[0][:, 0:1],
                           rsb[:, 0, 2 * pair:2 * pair + 1],
                           rsb[:, 1, 2 * pair:2 * pair + 1])
                a_b = feat.tile([128, N], F32, tag="feat", name=f"a1b_{pair}")
                nc.vector.tensor_copy(a_b[0:64, :], a_t[64:128, :])
                a_cur[(gA, 0)] = (a_t, 0, 64)
                a_cur[(gB, 0)] = (a_b, 0, 64)
            # combine packed stats: move bottom-64 accums to base 0, then reduce
            for pair in range(2):
                for k in range(2):
                    nc.vector.tensor_copy(rsb[0:64, k, 2 * pair + 1:2 * pair + 2],
                                          rsb[64:128, k, 2 * pair:2 * pair + 1])
            stat = sm.tile([128, 2 * CC], F32, tag=f"stat{li}", name=f"stat{li}")
            nc.vector.memset(stat[:], 0.0)
            for k in range(2):
                nc.vector.tensor_reduce(stat[0:64, k:k + 1], rsb[0:64, k, :],
                                        AX.X, ALU.add)
        else:
            for g in range(G):
                wrap = kp.tile([128, N], I16, tag="wrap", name=f"wrap{li}_{g}")
                topk_wrap(li, D, g, wrap, 0)
                nc.sync.dma_start(wrap[16:32, :], wrap[0:16, :])
                nc.vector.tensor_copy(wrap[32:64, :], wrap[0:32, :])
                nc.vector.tensor_copy(wrap[64:128, :], wrap[0:64, :])
                a_new = []
                for cc in range(CC):
                    cw = min(128, C - 128 * cc)
                    csl = slice(128 * cc, 128 * cc + cw)
                    vT = wk.tile([128, N], F32, tag="vT", name=f"vT{li}_{g}_{cc}")
                    uT = wk.tile([128, N], F32, tag="uT", name=f"uT{li}_{g}_{cc}")
                    uv_mats(li, D, g, csl, cw, vT, 0, uT, 0)
                    h = wk.tile([128, N], F32, tag="h", name=f"h{li}_{g}_{cc}")
                    gather_block(eng_rr, wrap, vT, uT, h, cw)
                    eng_rr += 1
                    a_t = feat.tile([128, N], F32, tag="feat", name=f"a{li + 1}_{g}_{cc}")
                    relu_stats(a_t, 0, h, 0, cw, rcol_prev,
                               badj[li][0:cw, cc:cc + 1],
                               rsb[0:cw, 2 * cc, g:g + 1],
                               rsb[0:cw, 2 * cc + 1, g:g + 1])
                    a_new.append((cc, a_t, cw))
                for cc, a_t, cw in a_new:
                    a_cur[(g, cc)] = (a_t, 0, cw)
            stat = sm.tile([128, 2 * CC], F32, tag=f"stat{li}", name=f"stat{li}")
            nc.vector.memset(stat[:], 0.0)
            for cc in range(CC):
                cw = min(128, C - 128 * cc)
                for k in range(2):
                    nc.vector.tensor_reduce(stat[0:cw, 2 * cc + k:2 * cc + k + 1],
                                            rsb[0:cw, 2 * cc + k, :], AX.X, ALU.add)
        stat_tiles[li] = stat

        # ---- AllReduce of [sum_a, sum_a2] per channel ----
        bi = dr.tile([128, 2 * CC], F32, tag=f"cc_in{li}", name=f"cc_in{li}")
        bo = dr.tile([128, 2 * CC], F32, tag=f"cc_out{li}", name=f"cc_out{li}")
        nc.gpsimd.dma_start(bi[:], stat[:])
        nc.gpsimd.collective_compute("AllReduce", ALU.add, replica_groups=replica,
                                     ins=[bi[:].opt()], outs=[bo[:].opt()])
        statg = sm.tile([128, 2 * CC], F32, tag=f"statg{li}", name=f"statg{li}")
        nc.gpsimd.dma_start(statg[:], bo[:])

        # mu, msn, r
        mu = sm.tile([128, CC], F32, tag=f"mu{li}", name=f"mu{li}")
        musq = sm.tile([128, CC], F32, tag=f"musq{li}", name=f"musq{li}")
        for cc in range(CC):
            nc.vector.tensor_scalar_mul(mu[:, cc:cc + 1], statg[:, 2 * cc:2 * cc + 1],
                                        1.0 / TOTAL_NODES)
        nc.vector.tensor_tensor(musq[:], mu[:], mu[:], ALU.mult)
        pr = ps_v.tile([1, 2], F32, tag="pvu")
        for cc in range(CC):
            nc.tensor.matmul(pr[:, 0:1], statg[:, 2 * cc + 1:2 * cc + 2], ones128[:],
                             start=(cc == 0), stop=(cc == CC - 1))
        for cc in range(CC):
            nc.tensor.matmul(pr[:, 1:2], musq[:, cc:cc + 1], ones128[:],
                             start=(cc == 0), stop=(cc == CC - 1))
        sc = sm.tile([1, 2], F32, tag=f"sc{li}", name=f"sc{li}")
        nc.scalar.activation(sc[:], pr[:], AF.Copy)
        rsc = sm.tile([1, 1], F32, tag=f"rsc{li}", name=f"rsc{li}")
        nc.vector.tensor_scalar(rsc[:], sc[:, 0:1], scalar1=1.0 / TOTAL_NODES,
                                scalar2=eps, op0=ALU.mult, op1=ALU.add)
        nc.vector.tensor_tensor(rsc[:], rsc[:], sc[:, 1:2], ALU.subtract)
        nc.vector.reciprocal(rsc[:], rsc[:])
        nc.scalar.activation(rsc[:], rsc[:], AF.Sqrt)
        prb = ps_v.tile([128, 1], F32, tag="pvu")
        nc.tensor.matmul(prb[:], ones1[:], rsc[:], start=True, stop=True)
        rcol = sm.tile([128, 1], F32, tag=f"rcol{li}", name=f"rcol{li}")
        nc.scalar.activation(rcol[:], prb[:], AF.Copy)
        rneg = sm.tile([128, 1], F32, tag=f"rneg{li}", name=f"rneg{li}")
        nc.scalar.activation(rneg[:], prb[:], AF.Copy, scale=-1.0)

        if li < 2:
            # next-layer folded bias: badj = b_{l+1} - r*Wa_{l+1}^T mu
            Dn, Cn = LAYERS[li + 1]
            CCn = _ccdiv(Cn)
            DCCn = _ccdiv(Dn)
            bt = sm.tile([128, CCn], F32, tag=f"badj{li + 1}", name=f"badj{li + 1}")
            for ccn in range(CCn):
                cwn = min(128, Cn - 128 * ccn)
                csln = slice(128 * ccn, 128 * ccn + cwn)
                pb = ps_v.tile([128, 1], F32, tag="pvu")
                for dc in range(DCCn):
                    nc.tensor.matmul(pb[0:cwn, :], W[("wa", li + 1, dc)][:, csln],
                                     mu[0:min(128, Dn - 128 * dc), dc:dc + 1],
                                     start=(dc == 0), stop=(dc == DCCn - 1))
                nc.scalar.activation(bt[0:cwn, ccn:ccn + 1], pb[0:cwn, :], AF.Identity,
                                     scale=rneg[0:cwn, :],
                                     bias=W[("b", li + 1)][0:cwn, ccn:ccn + 1])
            badj[li + 1] = bt
            rcol_prev = rcol
        else:
            # ---- head: g = r3*(max_n a3 - mu3); relu(Wl1^T g + bl1); Wl2 ----
            gmat = sm.tile([128, 2, G], F32, tag="gmat")
            for g in range(G):
                for cc in range(2):
                    t, lo, hi = a_cur[(g, cc)]
                    nc.vector.tensor_reduce(gmat[:, cc, g:g + 1], t[:], AX.X, ALU.max)
            # normalize pooled: gn = (gmat - mu3)*r
            for cc in range(2):
                nc.vector.tensor_scalar(gmat[:, cc, :], gmat[:, cc, :],
                                        scalar1=mu[:, cc:cc + 1], scalar2=rcol[:],
                                        op0=ALU.subtract, op1=ALU.mult)
            ph = ps_v.tile([64, G], F32, tag="pvu")
            for cc in range(2):
                nc.tensor.matmul(ph[:], wl1[:, cc, :], gmat[:, cc, :],
                                 start=(cc == 0), stop=(cc == 1))
            hh = sm.tile([64, G], F32, tag="hh")
            nc.scalar.activation(hh[:], ph[:], AF.Relu, bias=bl1[:])
            po = ps_v.tile([2, G], F32, tag="pvu")
            nc.tensor.matmul(po[:], wl2[:], hh[:], start=True, stop=True)
            oo = sm.tile([2, G], F32, tag="oo")
            nc.scalar.activation(oo[:], po[:], AF.Identity, bias=bl2[:])
            nc.sync.dma_start(outs["out"][:], oo[:])
    es.close()


def _host_weights(inputs):
    w = {}
    for li, (D, C) in enumerate(LAYERS):
        Wl = np.asarray(inputs[f"W{li + 1}"], dtype=np.float32)
        w[f"wab{li}"] = np.ascontiguousarray(Wl[:D] - Wl[D:])
        w[f"wb{li}"] = np.ascontiguousarray(Wl[D:])
        w[f"wa{li}"] = np.ascontiguousarray(Wl[:D])
        w[f"b{li}"] = np.ascontiguousarray(
            np.asarray(inputs[f"b{li + 1}"], dtype=np.float32).reshape(C, 1))
    w["wl1"] = np.ascontiguousarray(
        np.asarray(inputs["Wl1"], dtype=np.float32).reshape(2, 128, 64))
    w["bl1"] = np.asarray(inputs["bl1"], dtype=np.float32).reshape(64, 1).copy()
    w["wl2"] = np.ascontiguousarray(np.asarray(inputs["Wl2"], dtype=np.float32))
    w["bl2"] = np.asarray(inputs["bl2"], dtype=np.float32).reshape(2, 1).copy()
    return w


_CACHED = {}


def _get_module():
    if "nc" in _CACHED:
        return _CACHED["nc"]
    nc = bacc.Bacc("TRN2", target_bir_lowering=False, debug=False, num_devices=N_CORES)
    ins = {"pos": nc.dram_tensor("pos", (G, N, 2), F32, kind="ExternalInput")}
    for li, (D, C) in enumerate(LAYERS):
        ins[f"wab{li}"] = nc.dram_tensor(f"wab{li}", (D, C), F32, kind="ExternalInput")
        ins[f"wb{li}"] = nc.dram_tensor(f"wb{li}", (D, C), F32, kind="ExternalInput")
        ins[f"wa{li}"] = nc.dram_tensor(f"wa{li}", (D, C), F32, kind="ExternalInput")
        ins[f"b{li}"] = nc.dram_tensor(f"b{li}", (C, 1), F32, kind="ExternalInput")
    ins["wl1"] = nc.dram_tensor("wl1", (2, 128, 64), F32, kind="ExternalInput")
    ins["bl1"] = nc.dram_tensor("bl1", (64, 1), F32, kind="ExternalInput")
    ins["wl2"] = nc.dram_tensor("wl2", (64, 2), F32, kind="ExternalInput")
    ins["bl2"] = nc.dram_tensor("bl2", (2, 1), F32, kind="ExternalInput")
    outs = {"out": nc.dram_tensor("out", (2, G), F32, kind="ExternalOutput")}
    with tile.TileContext(nc) as tc:
        _build(tc, nc, ins, outs, n_cores=N_CORES)
    nc.compile()
    _CACHED["nc"] = nc
    return nc


def kernel(**inputs):
    pos = np.ascontiguousarray(np.asarray(inputs["pos"], dtype=np.float32))
    w = _host_weights(inputs)
    nc = _get_module()
    in_maps = []
    for core in range(N_CORES):
        m = {"pos": np.ascontiguousarray(pos[core * G:(core + 1) * G])}
        m.update(w)
        in_maps.append(m)
    res = run_bass_kernel_spmd(nc, in_maps, list(range(N_CORES)))
    outs = [res.results[c]["out"].T for c in range(N_CORES)]  # each [G, 2]
    return np.concatenate(outs, axis=0).astype(np.float32)


# revision 17
# speedup vs baseline: 1.2607x; 1.0317x over previous
# Trainium2 Bass kernel for nn_Edge_CNN (DynamicEdgeConv x3 + PairNorm + pool + MLP head).
#
# Data-parallel over the 32 graphs -> 8 NeuronCores x 4 graphs. PairNorm couples
# graphs only through per-channel mean + a scalar; those stats go through a tiny
# AllReduce whose result is folded into the NEXT layer's activation (scale r,
# bias b - r*Wa^T mu), so all heavy per-layer compute runs on unnormalized
# activations (kNN selection is invariant under the shared affine transform).
#
# Top-k selection per 128-row chunk packs the candidate index into the low 10
# mantissa bits of the (quantized) negative-distance fp32 ("keys"), so a single
# max8 pass yields values AND indices: stt-keys -> max8 -> match_replace ->
# max8 gives ranks 1..16. Slot-index rows are produced by a PE transpose of the
# top-16 keys plus one fused AND-extract; the kNN gather streams 16 slots/node
# but the slot-max reduce reads only slots 0..10 (k=10 exact). Slot reduction is
# split between DVE (tensor_reduce) and GpSimd (tensor_tensor max tree) to
# balance engine load.
#
# kernel(**inputs) takes FULL unsharded inputs, returns the FULL [32, 2].

import numpy as np
from contextlib import ExitStack

import concourse.bass as bass
import concourse.bacc as bacc
import concourse.mybir as mybir
import concourse.tile as tile
from concourse.bass_utils import run_bass_kernel_spmd

N = 1024
B_TOTAL = 32
N_CORES = 8
G = B_TOTAL // N_CORES
F32 = mybir.dt.float32
U32 = mybir.dt.uint32
U16 = mybir.dt.uint16
I16 = mybir.dt.int16
AF = mybir.ActivationFunctionType
ALU = mybir.AluOpType
AX = mybir.AxisListType
NCHUNK = N // 128
LAYERS = [(2, 64), (64, 128), (128, 256)]  # (D_in, C_out)
KSLOT = 16   # gather slots per node (ranks 1..16; reduce uses first 10)
KTOP = 10
NQ = 4       # gather split: quarters per (unit, cc)
HQ = N // NQ
DVE_RED_EVERY = 8   # every k-th quarter reduces on DVE; rest on Pool tree


def _ccdiv(c):
    return (c + 127) // 128


def _build(tc, nc, ins, outs, n_cores, eps=1e-5):
    TOTAL_NODES = float(B_TOTAL * N)
    replica = [list(range(n_cores))]

    es = ExitStack()
    sb = es.enter_context(tc.tile_pool(name="sb", bufs=1))
    feat = es.enter_context(tc.tile_pool(name="feat", bufs=14))
    kp = es.enter_context(tc.tile_pool(name="kp", bufs=2))
    wk = es.enter_context(tc.tile_pool(name="wk", bufs=2))
    sm = es.enter_context(tc.tile_pool(name="sm", bufs=4))
    gt = es.enter_context(tc.tile_pool(name="gt", bufs=2))
    ps_g = es.enter_context(tc.tile_pool(name="ps_g", bufs=2, space="PSUM"))
    ps_v = es.enter_context(tc.tile_pool(name="ps_v", bufs=2, space="PSUM"))
    dr = es.enter_context(tc.tile_pool(name="dr", bufs=1, space="DRAM"))

    # ---- one-time constants ----
    ones1 = sb.tile([1, 128], F32, tag="ones1")
    nc.vector.memset(ones1[:], 1.0)
    ones128 = sb.tile([128, 1], F32, tag="ones128")
    nc.vector.memset(ones128[:], 1.0)
    ones_tn = sb.tile([128, 1], F32, tag="ones_tn")
    nc.vector.memset(ones_tn[:], 1.0 / TOTAL_NODES)
    ones_neg = sb.tile([128, 1], F32, tag="ones_neg")
    nc.vector.memset(ones_neg[:], -1.0)
    iota = sb.tile([128, N], U32, tag="iota")
    nc.gpsimd.iota(iota[:], pattern=[[1, N]], base=0, channel_multiplier=0)
    kmask = sb.tile([128, 1], U32, tag="kmask")
    nc.vector.memset(kmask[:], 0xFFFFFC00)
    kmask10 = sb.tile([128, 1], U16, tag="kmask10")
    nc.vector.memset(kmask10[:], 0x3FF)
    epsT = sb.tile([1, 1], F32, tag="epsT")
    nc.vector.memset(epsT[:], eps)
    eye = sb.tile([128, 128], F32, tag="eye")
    nc.sync.dma_start(eye[:], ins["eye"][:])
    ktops = [sb.tile([128, 16], F32, tag=f"ktop{i}", name=f"ktop{i}")
             for i in range(2)]

    # ---- weights ----
    W = {}
    for li, (D, C) in enumerate(LAYERS):
        DCC = _ccdiv(D)
        for nm in ("wab", "wb", "wa"):
            for dc in range(DCC):
                dw = min(128, D - 128 * dc)
                t = sb.tile([dw, C], F32, tag=f"{nm}{li}_{dc}", name=f"{nm}{li}_{dc}")
                nc.sync.dma_start(t[:], ins[f"{nm}{li}"][128 * dc:128 * dc + dw, :])
                W[(nm, li, dc)] = t
        cw0 = min(C, 128)
        t = sb.tile([cw0, _ccdiv(C)], F32, tag=f"b{li}", name=f"b{li}")
        nc.sync.dma_start(t[:], ins[f"b{li}"][:].rearrange("(cc p) one -> p (cc one)",
                                                           p=cw0))
        W[("b", li)] = t
    wl1 = sb.tile([128, 2, 64], F32, tag="wl1")
    nc.sync.dma_start(wl1[:], ins["wl1"][:].rearrange("cc p c -> p cc c"))
    wl2 = sb.tile([64, 2], F32, tag="wl2")
    nc.sync.dma_start(wl2[:], ins["wl2"][:])
    bl1 = sb.tile([64, 1], F32, tag="bl1")
    nc.sync.dma_start(bl1[:], ins["bl1"][:])
    bl2 = sb.tile([2, 1], F32, tag="bl2")
    nc.sync.dma_start(bl2[:], ins["bl2"][:])

    # layer-0 packed bias [b0; b0] and scale r=1
    b0p = sb.tile([128, 1], F32, tag="b0p")
    nc.vector.tensor_copy(b0p[0:64, :], W[("b", 0)][:, 0:1])
    nc.vector.tensor_copy(b0p[64:128, :], W[("b", 0)][:, 0:1])
    ones_r = sb.tile([128, 1], F32, tag="ones_r")
    nc.vector.memset(ones_r[:], 1.0)

    # ---- load pos ----
    a_cur = {}  # (g, cc) -> (tile, rowlo, rowhi)
    for g in range(G):
        t = feat.tile([128, N], F32, tag="feat", name=f"a0_{g}")
        nc.sync.dma_start(t[0:2, :], ins["pos"][g, :, :].rearrange("j d -> d j"))
        a_cur[(g, 0)] = (t, 0, 2)

    qctr = [0]

    def topk_wrap(li, D, g, wrap, wrow):
        """kNN top-16 index rows into wrap[wrow:wrow+16, :] for graph g."""
        DCC = _ccdiv(D)
        srcs = [a_cur[(g, dc)] for dc in range(DCC)]

        sq = wk.tile([128, N], F32, tag="sq")
        psq = ps_v.tile([1, N], F32, tag="pvu")
        for dc in range(DCC):
            t, lo, hi = srcs[dc]
            nc.scalar.activation(sq[0:hi - lo, :], t[lo:hi, :], AF.Square)
            for b in range(2):
                sl = slice(512 * b, 512 * (b + 1))
                nc.tensor.matmul(psq[:, sl], ones128[0:hi - lo, :], sq[0:hi - lo, sl],
                                 start=(dc == 0), stop=(dc == DCC - 1))
        rq = wk.tile([1, N], F32, tag="rq")
        nc.scalar.activation(rq[:], psq[:], AF.Copy, scale=-0.5)

        for c in range(NCHUNK):
            csl = slice(128 * c, 128 * (c + 1))
            pg = ps_g.tile([128, N], F32, tag="gram")
            for b in range(2):
                sl = slice(512 * b, 512 * (b + 1))
                for dc in range(DCC):
                    t, lo, hi = srcs[dc]
                    nc.tensor.matmul(pg[:, sl], t[lo:hi, csl], t[lo:hi, sl],
                                     start=(dc == 0), stop=False)
                nc.tensor.matmul(pg[:, sl], ones1[:], rq[:, sl], start=False, stop=True)
            keys = kp.tile([128, N], F32, tag="keys")
            nc.vector.scalar_tensor_tensor(keys[:].bitcast(U32), pg[:].bitcast(U32),
                                           kmask[:], iota[:],
                                           op0=ALU.bitwise_and, op1=ALU.bitwise_or)
            ktop = ktops[c % 2]
            nc.vector.max(ktop[:, 0:8], keys[:])
            nc.vector.match_replace(keys[:], ktop[:, 0:8], keys[:], -3.0e38)
            nc.vector.max(ktop[:, 8:16], keys[:])
            pT = ps_v.tile([16, 128], F32, tag="pvu")
            nc.tensor.transpose(pT[:], ktop[:], eye[:])
            nc.vector.tensor_scalar(
                wrap[wrow:wrow + 16, csl].bitcast(U16),
                pT[:].bitcast(U16).rearrange("p (s two) -> p s two", two=2)[:, :, 0],
                scalar1=kmask10[0:16, :], scalar2=None,
                op0=ALU.bitwise_and)

    def pool_maxtree(h, cw, gout, qsl):
        """h[0:cw,qsl] = max over slots 0..10 of gout (GpSimd TT tree)."""
        g3 = gout[:].rearrange("p (i s) -> p i s", s=KSLOT)
        t5 = gt.tile([128, HQ, 5], F32, tag="t5")
        nc.gpsimd.tensor_tensor(t5[:], g3[:, :, 0:5], g3[:, :, 5:10], ALU.max)
        t2 = gt.tile([128, HQ, 2], F32, tag="t2")
        nc.gpsimd.tensor_tensor(t2[:], t5[:, :, 0:2], t5[:, :, 2:4], ALU.max)
        nc.gpsimd.tensor_tensor(h[0:cw, qsl], t2[0:cw, :, 0], t2[0:cw, :, 1], ALU.max)
        nc.gpsimd.tensor_tensor(h[0:cw, qsl], h[0:cw, qsl], t5[0:cw, :, 4], ALU.max)

    def gather_block(wrap, vT, uT, h, cw):
        """h[0:cw,:] = max over slots 0..10 of gathered vT + uT (raw pre-act)."""
        for q in range(NQ):
            qsl = slice(HQ * q, HQ * (q + 1))
            gout = gt.tile([128, KSLOT * HQ], F32, tag="gout", name=f"gout{q}")
            nc.gpsimd.ap_gather(gout[:], vT[:], wrap[:, qsl],
                                channels=128, num_elems=N, d=1, num_idxs=KSLOT * HQ)
            nc.vector.tensor_reduce(h[0:cw, qsl],
                                    gout[0:cw, :].rearrange("p (i s) -> p i s",
                                                            s=KSLOT)[:, :, 0:KTOP],
                                    AX.X, ALU.max)
            qctr[0] += 1
        nc.gpsimd.tensor_tensor(h[0:cw, :], h[0:cw, :], uT[0:cw, :], ALU.add)

    def uv_mats(li, D, g, csl, cw, vT, vrow, uT, urow):
        DCC = _ccdiv(D)
        srcs = [a_cur[(g, dc)] for dc in range(DCC)]
        pv = ps_v.tile([128, N], F32, tag="pvu")
        for b in range(2):
            sl = slice(512 * b, 512 * (b + 1))
            for dc in range(DCC):
                t, lo, hi = srcs[dc]
                nc.tensor.matmul(pv[0:cw, sl], W[("wb", li, dc)][:, csl], t[lo:hi, sl],
                                 start=(dc == 0), stop=(dc == DCC - 1))
        nc.scalar.activation(vT[vrow:vrow + cw, :], pv[0:cw, :], AF.Copy)
        pu = ps_v.tile([128, N], F32, tag="pvu")
        for b in range(2):
            sl = slice(512 * b, 512 * (b + 1))
            for dc in range(DCC):
                t, lo, hi = srcs[dc]
                nc.tensor.matmul(pu[0:cw, sl], W[("wab", li, dc)][:, csl], t[lo:hi, sl],
                                 start=(dc == 0), stop=(dc == DCC - 1))
        nc.scalar.activation(uT[urow:urow + cw, :], pu[0:cw, :], AF.Copy)

    def relu_stats(a_t, arow, h, hrow, cw, rcol, badj_ap, rs, qs):
        nc.scalar.activation(a_t[arow:arow + cw, :], h[hrow:hrow + cw, :], AF.Relu,
                             scale=rcol, bias=badj_ap, accum_out=rs)
        sqh = wk.tile([128, N], F32, tag="sqh")
        nc.scalar.activation(sqh[0:cw, :], a_t[arow:arow + cw, :], AF.Square,
                             accum_out=qs)

    # ---- normalization state ----
    norm = {0: (ones_r, {0: b0p})}  # li -> (rcol of layer li-1, badj tile map)

    def make_fin(li, statg, CC, C):
        """Post-collective finalize for layer li: mu, r, next badj (Act+PE only)."""
        def fin():
            mu = sm.tile([128, CC], F32, tag=f"mu{li}", name=f"mu{li}")
            nc.scalar.activation(
                mu[:], statg[:].rearrange("p (c k) -> p c k", k=2)[:, :, 0],
                AF.Copy, scale=1.0 / TOTAL_NODES)
            musq = sm.tile([128, CC], F32, tag=f"musq{li}", name=f"musq{li}")
            nc.scalar.activation(musq[:], mu[:], AF.Square)
            pr = ps_v.tile([1, 1], F32, tag="pvu")
            for cc in range(CC):
                nc.tensor.matmul(pr[:], statg[:, 2 * cc + 1:2 * cc + 2], ones_tn[:],
                                 start=(cc == 0), stop=False)
            for cc in range(CC):
                nc.tensor.matmul(pr[:], musq[:, cc:cc + 1], ones_neg[:],
                                 start=False, stop=(cc == CC - 1))
            rsq = sm.tile([1, 1], F32, tag=f"rsq{li}", name=f"rsq{li}")
            nc.scalar.activation(rsq[:], pr[:], AF.Sqrt, bias=epsT[:])
            rsc = sm.tile([1, 1], F32, tag=f"rsc{li}", name=f"rsc{li}")
            nc.vector.reciprocal(rsc[:], rsq[:])
            prb = ps_v.tile([128, 1], F32, tag="pvu")
            nc.tensor.matmul(prb[:], ones1[:], rsc[:], start=True, stop=True)
            rcol = sm.tile([128, 1], F32, tag=f"rcol{li}", name=f"rcol{li}")
            nc.scalar.activation(rcol[:], prb[:], AF.Copy)
            rneg = sm.tile([128, 1], F32, tag=f"rneg{li}", name=f"rneg{li}")
            nc.scalar.activation(rneg[:], prb[:], AF.Copy, scale=-1.0)
            if li < 2:
                Dn, Cn = LAYERS[li + 1]
                CCn = _ccdiv(Cn)
                DCCn = _ccdiv(Dn)
                bt = sm.tile([128, CCn], F32, tag=f"badj{li + 1}",
                             name=f"badj{li + 1}")
                for ccn in range(CCn):
                    cwn = min(128, Cn - 128 * ccn)
                    csln = slice(128 * ccn, 128 * ccn + cwn)
                    pb = ps_v.tile([128, 1], F32, tag="pvu")
                    for dc in range(DCCn):
                        nc.tensor.matmul(pb[0:cwn, :], W[("wa", li + 1, dc)][:, csln],
                                         mu[0:min(128, Dn - 128 * dc), dc:dc + 1],
                                         start=(dc == 0), stop=(dc == DCCn - 1))
                    nc.scalar.activation(bt[0:cwn, ccn:ccn + 1], pb[0:cwn, :],
                                         AF.Identity, scale=rneg[0:cwn, :],
                                         bias=W[("b", li + 1)][0:cwn, ccn:ccn + 1])
                norm[li + 1] = (rcol, {cc: bt for cc in range(CCn)})
            else:
                norm["head"] = (rcol, mu)
        return fin

    pending_fin = [None]

    def run_fin():
        if pending_fin[0] is not None:
            pending_fin[0]()
            pending_fin[0] = None

    for li, (D, C) in enumerate(LAYERS):
        CC = _ccdiv(C)
        rsb = sm.tile([128, 2 * CC, G], F32, tag=f"rsb{li}", name=f"rsb{li}")

        if li == 0:
            for pair in range(2):
                gA, gB = 2 * pair, 2 * pair + 1
                wrap = kp.tile([128, N], I16, tag="wrap", name=f"wrap0_{pair}")
                topk_wrap(0, D, gA, wrap, 0)
                topk_wrap(0, D, gB, wrap, 64)
                nc.sync.dma_start(wrap[16:32, :], wrap[0:16, :])
                nc.vector.tensor_copy(wrap[32:64, :], wrap[0:32, :])
                nc.sync.dma_start(wrap[80:96, :], wrap[64:80, :])
                nc.vector.tensor_copy(wrap[96:128, :], wrap[64:96, :])
                vT = wk.tile([128, N], F32, tag="vT", name=f"vT0_{pair}")
                uT = wk.tile([128, N], F32, tag="uT", name=f"uT0_{pair}")
                uv_mats(0, D, gA, slice(0, 64), 64, vT, 0, uT, 0)
                uv_mats(0, D, gB, slice(0, 64), 64, vT, 64, uT, 64)
                h = wk.tile([128, N], F32, tag="h", name=f"h0_{pair}")
                gather_block(wrap, vT, uT, h, 128)
                rcol0, badj0 = norm[0]
                a_t = feat.tile([128, N], F32, tag="feat", name=f"a1_{pair}")
                relu_stats(a_t, 0, h, 0, 128, rcol0, badj0[0][:, 0:1],
                           rsb[:, 0, 2 * pair:2 * pair + 1],
                           rsb[:, 1, 2 * pair:2 * pair + 1])
                a_b = feat.tile([128, N], F32, tag="feat", name=f"a1b_{pair}")
                nc.vector.tensor_copy(a_b[0:64, :], a_t[64:128, :])
                a_cur[(gA, 0)] = (a_t, 0, 64)
                a_cur[(gB, 0)] = (a_b, 0, 64)
            for pair in range(2):
                for k in range(2):
                    nc.vector.tensor_copy(rsb[0:64, k, 2 * pair + 1:2 * pair + 2],
                                          rsb[64:128, k, 2 * pair:2 * pair + 1])
            stat = sm.tile([128, 2 * CC], F32, tag=f"stat{li}", name=f"stat{li}")
            nc.vector.memset(stat[:], 0.0)
            for k in range(2):
                nc.vector.tensor_reduce(stat[0:64, k:k + 1], rsb[0:64, k, :],
                                        AX.X, ALU.add)
        else:
            for g in range(G):
                wrap = kp.tile([128, N], I16, tag="wrap", name=f"wrap{li}_{g}")
                topk_wrap(li, D, g, wrap, 0)
                if g == 0:
                    run_fin()   # overlap prev-layer collective with this topk
                nc.sync.dma_start(wrap[16:32, :], wrap[0:16, :])
                nc.vector.tensor_copy(wrap[32:64, :], wrap[0:32, :])
                nc.vector.tensor_copy(wrap[64:128, :], wrap[0:64, :])
                rcol_p, badj_p = norm[li]
                a_new = []
                for cc in range(CC):
                    cw = min(128, C - 128 * cc)
                    csl = slice(128 * cc, 128 * cc + cw)
                    vT = wk.tile([128, N], F32, tag="vT", name=f"vT{li}_{g}_{cc}")
                    uT = wk.tile([128, N], F32, tag="uT", name=f"uT{li}_{g}_{cc}")
                    uv_mats(li, D, g, csl, cw, vT, 0, uT, 0)
                    h = wk.tile([128, N], F32, tag="h", name=f"h{li}_{g}_{cc}")
                    gather_block(wrap, vT, uT, h, cw)
                    a_t = feat.tile([128, N], F32, tag="feat",
                                    name=f"a{li + 1}_{g}_{cc}")
                    relu_stats(a_t, 0, h, 0, cw, rcol_p, badj_p[cc][0:cw, cc:cc + 1],
                               rsb[0:cw, 2 * cc, g:g + 1],
                               rsb[0:cw, 2 * cc + 1, g:g + 1])
                    a_new.append((cc, a_t, cw))
                for cc, a_t, cw in a_new:
                    a_cur[(g, cc)] = (a_t, 0, cw)
            stat = sm.tile([128, 2 * CC], F32, tag=f"stat{li}", name=f"stat{li}")
            nc.vector.memset(stat[:], 0.0)
            for cc in range(CC):
                cw = min(128, C - 128 * cc)
                for k in range(2):
                    nc.vector.tensor_reduce(stat[0:cw, 2 * cc + k:2 * cc + k + 1],
                                            rsb[0:cw, 2 * cc + k, :], AX.X, ALU.add)

        # ---- AllReduce of [sum_a, sum_a2] per channel ----
        bi = dr.tile([128, 2 * CC], F32, tag=f"cc_in{li}", name=f"cc_in{li}")
        bo = dr.tile([128, 2 * CC], F32, tag=f"cc_out{li}", name=f"cc_out{li}")
        nc.scalar.dma_start(bi[:], stat[:])
        nc.gpsimd.collective_compute("AllReduce", ALU.add, replica_groups=replica,
                                     ins=[bi[:].opt()], outs=[bo[:].opt()])
        statg = sm.tile([128, 2 * CC], F32, tag=f"statg{li}", name=f"statg{li}")
        nc.scalar.dma_start(statg[:], bo[:])
        pending_fin[0] = make_fin(li, statg, CC, C)

    # ---- head ----
    gmat = sm.tile([128, 2, G], F32, tag="gmat")
    for g in range(G):
        for cc in range(2):
            t, lo, hi = a_cur[(g, cc)]
            nc.vector.tensor_reduce(gmat[:, cc, g:g + 1], t[:], AX.X, ALU.max)
    run_fin()
    rcol3, mu3 = norm["head"]
    for cc in range(2):
        nc.vector.tensor_scalar(gmat[:, cc, :], gmat[:, cc, :],
                                scalar1=mu3[:, cc:cc + 1], scalar2=rcol3[:],
                                op0=ALU.subtract, op1=ALU.mult)
    ph = ps_v.tile([64, G], F32, tag="pvu")
    for cc in range(2):
        nc.tensor.matmul(ph[:], wl1[:, cc, :], gmat[:, cc, :],
                         start=(cc == 0), stop=(cc == 1))
    hh = sm.tile([64, G], F32, tag="hh")
    nc.scalar.activation(hh[:], ph[:], AF.Relu, bias=bl1[:])
    po = ps_v.tile([2, G], F32, tag="pvu")
    nc.tensor.matmul(po[:], wl2[:], hh[:], start=True, stop=True)
    oo = sm.tile([2, G], F32, tag="oo")
    nc.scalar.activation(oo[:], po[:], AF.Identity, bias=bl2[:])
    nc.sync.dma_start(outs["out"][:], oo[:])
    es.close()


def _host_weights(inputs):
    w = {}
    for li, (D, C) in enumerate(LAYERS):
        Wl = np.asarray(inputs[f"W{li + 1}"], dtype=np.float32)
        w[f"wab{li}"] = np.ascontiguousarray(Wl[:D] - Wl[D:])
        w[f"wb{li}"] = np.ascontiguousarray(Wl[D:])
        w[f"wa{li}"] = np.ascontiguousarray(Wl[:D])
        w[f"b{li}"] = np.ascontiguousarray(
            np.asarray(inputs[f"b{li + 1}"], dtype=np.float32).reshape(C, 1))
    w["wl1"] = np.ascontiguousarray(
        np.asarray(inputs["Wl1"], dtype=np.float32).reshape(2, 128, 64))
    w["bl1"] = np.asarray(inputs["bl1"], dtype=np.float32).reshape(64, 1).copy()
    w["wl2"] = np.ascontiguousarray(np.asarray(inputs["Wl2"], dtype=np.float32))
    w["bl2"] = np.asarray(inputs["bl2"], dtype=np.float32).reshape(2, 1).copy()
    w["eye"] = np.eye(128, dtype=np.float32)
    return w


_CACHED = {}


def _get_module():
    if "nc" in _CACHED:
        return _CACHED["nc"]
    nc = bacc.Bacc("TRN2", target_bir_lowering=False, debug=False, num_devices=N_CORES)
    ins = {"pos": nc.dram_tensor("pos", (G, N, 2), F32, kind="ExternalInput")}
    for li, (D, C) in enumerate(LAYERS):
        ins[f"wab{li}"] = nc.dram_tensor(f"wab{li}", (D, C), F32, kind="ExternalInput")
        ins[f"wb{li}"] = nc.dram_tensor(f"wb{li}", (D, C), F32, kind="ExternalInput")
        ins[f"wa{li}"] = nc.dram_tensor(f"wa{li}", (D, C), F32, kind="ExternalInput")
        ins[f"b{li}"] = nc.dram_tensor(f"b{li}", (C, 1), F32, kind="ExternalInput")
    ins["wl1"] = nc.dram_tensor("wl1", (2, 128, 64), F32, kind="ExternalInput")
    ins["bl1"] = nc.dram_tensor("bl1", (64, 1), F32, kind="ExternalInput")
    ins["wl2"] = nc.dram_tensor("wl2", (64, 2), F32, kind="ExternalInput")
    ins["bl2"] = nc.dram_tensor("bl2", (2, 1), F32, kind="ExternalInput")
    ins["eye"] = nc.dram_tensor("eye", (128, 128), F32, kind="ExternalInput")
    outs = {"out": nc.dram_tensor("out", (2, G), F32, kind="ExternalOutput")}
    with tile.TileContext(nc) as tc:
        _build(tc, nc, ins, outs, n_cores=N_CORES)
    nc.compile()
    _CACHED["nc"] = nc
    return nc


def kernel(**inputs):
    pos = np.ascontiguousarray(np.asarray(inputs["pos"], dtype=np.float32))
    w = _host_weights(inputs)
    nc = _get_module()
    in_maps = []
    for core in range(N_CORES):
        m = {"pos": np.ascontiguousarray(pos[core * G:(core + 1) * G])}
        m.update(w)
        in_maps.append(m)
    res = run_bass_kernel_spmd(nc, in_maps, list(range(N_CORES)))
    outs = [res.results[c]["out"].T for c in range(N_CORES)]  # each [G, 2]
    return np.concatenate(outs, axis=0).astype(np.float32)


# revision 21
# speedup vs baseline: 1.3685x; 1.0856x over previous
# Trainium2 Bass kernel for nn_Edge_CNN (DynamicEdgeConv x3 + PairNorm + pool + MLP head).
#
# Data-parallel over the 32 graphs -> 8 NeuronCores x 4 graphs. PairNorm couples
# graphs only through per-channel mean + a scalar; those stats go through a tiny
# AllReduce whose result is folded into the NEXT layer's activation (scale r,
# bias b - r*Wa^T mu), so all heavy per-layer compute runs on unnormalized
# activations (kNN selection is invariant under the shared affine transform).
#
# Top-k selection per 128-row chunk packs the candidate index into the low 10
# mantissa bits of the (quantized) negative-distance fp32 ("keys"):
# stt-keys -> max8 -> match_replace -> max8 gives ranks 1..16. Slot-index rows
# come from a PE transpose of the top-16 keys plus one fused AND-extract; the
# kNN gather streams 16 slots/node but the slot-max reduce reads only slots
# 0..10 (k=10 exact).
#
# Emission is software-pipelined: each unit's gather/reduce/activation phase is
# woven into the NEXT unit's distance/top-k chunk loop so DVE never idles on
# Pool gathers, and PE transposes are deferred one chunk so the PE queue never
# blocks on DVE.
#
# kernel(**inputs) takes FULL unsharded inputs, returns the FULL [32, 2].

import numpy as np
from contextlib import ExitStack

import concourse.bass as bass
import concourse.bacc as bacc
import concourse.mybir as mybir
import concourse.tile as tile
from concourse.bass_utils import run_bass_kernel_spmd

N = 1024
B_TOTAL = 32
N_CORES = 8
G = B_TOTAL // N_CORES
F32 = mybir.dt.float32
U32 = mybir.dt.uint32
U16 = mybir.dt.uint16
I16 = mybir.dt.int16
AF = mybir.ActivationFunctionType
ALU = mybir.AluOpType
AX = mybir.AxisListType
NCHUNK = N // 128
LAYERS = [(2, 64), (64, 128), (128, 256)]  # (D_in, C_out)
KSLOT = 16
KTOP = 10
NQ = 4
HQ = N // NQ


def _ccdiv(c):
    return (c + 127) // 128


def _build(tc, nc, ins, outs, n_cores, eps=1e-5):
    TOTAL_NODES = float(B_TOTAL * N)
    replica = [list(range(n_cores))]

    es = ExitStack()
    sb = es.enter_context(tc.tile_pool(name="sb", bufs=1))
    feat = es.enter_context(tc.tile_pool(name="feat", bufs=10))
    kp = es.enter_context(tc.tile_pool(name="kp", bufs=2))
    wk = es.enter_context(tc.tile_pool(name="wk", bufs=2))
    sm = es.enter_context(tc.tile_pool(name="sm", bufs=4))
    gt = es.enter_context(tc.tile_pool(name="gt", bufs=4))
    ps_g = es.enter_context(tc.tile_pool(name="ps_g", bufs=1, space="PSUM"))
    ps_v = es.enter_context(tc.tile_pool(name="ps_v", bufs=2, space="PSUM"))
    dr = es.enter_context(tc.tile_pool(name="dr", bufs=1, space="DRAM"))

    # ---- one-time constants ----
    ones1 = sb.tile([1, 128], F32, tag="ones1")
    nc.vector.memset(ones1[:], 1.0)
    ones128 = sb.tile([128, 1], F32, tag="ones128")
    nc.vector.memset(ones128[:], 1.0)
    ones_tn = sb.tile([128, 1], F32, tag="ones_tn")
    nc.vector.memset(ones_tn[:], 1.0 / TOTAL_NODES)
    ones_neg = sb.tile([128, 1], F32, tag="ones_neg")
    nc.vector.memset(ones_neg[:], -1.0)
    iota = sb.tile([128, N], U32, tag="iota")
    nc.gpsimd.iota(iota[:], pattern=[[1, N]], base=0, channel_multiplier=0)
    kmask = sb.tile([128, 1], U32, tag="kmask")
    nc.vector.memset(kmask[:], 0xFFFFFC00)
    kmask10 = sb.tile([128, 1], U16, tag="kmask10")
    nc.vector.memset(kmask10[:], 0x3FF)
    epsT = sb.tile([1, 1], F32, tag="epsT")
    nc.vector.memset(epsT[:], eps)
    eye = sb.tile([128, 128], F32, tag="eye")
    nc.sync.dma_start(eye[:], ins["eye"][:])
    ktops = [sb.tile([128, 16], F32, tag=f"ktop{i}", name=f"ktop{i}")
             for i in range(2)]

    # ---- weights ----
    W = {}
    for li, (D, C) in enumerate(LAYERS):
        DCC = _ccdiv(D)
        for nm in ("wab", "wb", "wa"):
            for dc in range(DCC):
                dw = min(128, D - 128 * dc)
                t = sb.tile([dw, C], F32, tag=f"{nm}{li}_{dc}", name=f"{nm}{li}_{dc}")
                nc.sync.dma_start(t[:], ins[f"{nm}{li}"][128 * dc:128 * dc + dw, :])
                W[(nm, li, dc)] = t
        cw0 = min(C, 128)
        t = sb.tile([cw0, _ccdiv(C)], F32, tag=f"b{li}", name=f"b{li}")
        nc.sync.dma_start(t[:], ins[f"b{li}"][:].rearrange("(cc p) one -> p (cc one)",
                                                           p=cw0))
        W[("b", li)] = t
    wl1 = sb.tile([128, 2, 64], F32, tag="wl1")
    nc.sync.dma_start(wl1[:], ins["wl1"][:].rearrange("cc p c -> p cc c"))
    wl2 = sb.tile([64, 2], F32, tag="wl2")
    nc.sync.dma_start(wl2[:], ins["wl2"][:])
    bl1 = sb.tile([64, 1], F32, tag="bl1")
    nc.sync.dma_start(bl1[:], ins["bl1"][:])
    bl2 = sb.tile([2, 1], F32, tag="bl2")
    nc.sync.dma_start(bl2[:], ins["bl2"][:])

    b0p = sb.tile([128, 1], F32, tag="b0p")
    nc.vector.tensor_copy(b0p[0:64, :], W[("b", 0)][:, 0:1])
    nc.vector.tensor_copy(b0p[64:128, :], W[("b", 0)][:, 0:1])
    ones_r = sb.tile([128, 1], F32, tag="ones_r")
    nc.vector.memset(ones_r[:], 1.0)

    # ---- pos ----
    a_cur = {}
    for g in range(G):
        t = feat.tile([128, N], F32, tag="feat", name=f"a0_{g}")
        nc.sync.dma_start(t[0:2, :], ins["pos"][g, :, :].rearrange("j d -> d j"))
        a_cur[(g, 0)] = (t, 0, 2)

    # ---- task weaving ----
    pending = []

    def weave():
        if pending:
            pending.pop(0)()

    def drain():
        while pending:
            pending.pop(0)()

    def topk_wrap(li, D, g, wrap, wrow):
        """kNN top-16 slot rows into wrap[wrow:wrow+16, :]; weaves one pending
        task after each chunk; PE transpose deferred one chunk."""
        DCC = _ccdiv(D)
        srcs = [a_cur[(g, dc)] for dc in range(DCC)]

        sq = wk.tile([128, N], F32, tag="sq")
        psq = ps_v.tile([1, N], F32, tag="pvu")
        for dc in range(DCC):
            t, lo, hi = srcs[dc]
            nc.scalar.activation(sq[0:hi - lo, :], t[lo:hi, :], AF.Square)
            for b in range(2):
                sl = slice(512 * b, 512 * (b + 1))
                nc.tensor.matmul(psq[:, sl], ones128[0:hi - lo, :], sq[0:hi - lo, sl],
                                 start=(dc == 0), stop=(dc == DCC - 1))
        rq = wk.tile([1, N], F32, tag="rq")
        nc.scalar.activation(rq[:], psq[:], AF.Copy, scale=-0.5)

        def trans_extract(c):
            csl = slice(128 * c, 128 * (c + 1))
            pT = ps_v.tile([16, 128], F32, tag="pvu")
            nc.tensor.transpose(pT[:], ktops[c % 2][:], eye[:])
            nc.vector.tensor_scalar(
                wrap[wrow:wrow + 16, csl].bitcast(U16),
                pT[:].bitcast(U16).rearrange("p (s two) -> p s two", two=2)[:, :, 0],
                scalar1=kmask10[0:16, :], scalar2=None, op0=ALU.bitwise_and)

        for c in range(NCHUNK):
            csl = slice(128 * c, 128 * (c + 1))
            pg = ps_g.tile([128, N], F32, tag="gram")
            for b in range(2):
                sl = slice(512 * b, 512 * (b + 1))
                for dc in range(DCC):
                    t, lo, hi = srcs[dc]
                    nc.tensor.matmul(pg[:, sl], t[lo:hi, csl], t[lo:hi, sl],
                                     start=(dc == 0), stop=False)
                nc.tensor.matmul(pg[:, sl], ones1[:], rq[:, sl], start=False, stop=True)
            keys = kp.tile([128, N], F32, tag="keys")
            nc.vector.scalar_tensor_tensor(keys[:].bitcast(U32), pg[:].bitcast(U32),
                                           kmask[:], iota[:],
                                           op0=ALU.bitwise_and, op1=ALU.bitwise_or)
            ktop = ktops[c % 2]
            nc.vector.max(ktop[:, 0:8], keys[:])
            nc.vector.match_replace(keys[:], ktop[:, 0:8], keys[:], -3.0e38)
            nc.vector.max(ktop[:, 8:16], keys[:])
            if c > 0:
                trans_extract(c - 1)
            weave()
        trans_extract(NCHUNK - 1)

    def replicate_wrap(wrap, base, full):
        nc.sync.dma_start(wrap[base + 16:base + 32, :], wrap[base:base + 16, :])
        nc.vector.tensor_copy(wrap[base + 32:base + 64, :], wrap[base:base + 32, :])
        if full:
            nc.vector.tensor_copy(wrap[64:128, :], wrap[0:64, :])

    def uv_mats(li, D, g, csl, cw, vT, vrow, uT, urow):
        DCC = _ccdiv(D)
        srcs = [a_cur[(g, dc)] for dc in range(DCC)]
        pv = ps_v.tile([128, N], F32, tag="pvu")
        for b in range(2):
            sl = slice(512 * b, 512 * (b + 1))
            for dc in range(DCC):
                t, lo, hi = srcs[dc]
                nc.tensor.matmul(pv[0:cw, sl], W[("wb", li, dc)][:, csl], t[lo:hi, sl],
                                 start=(dc == 0), stop=(dc == DCC - 1))
        nc.scalar.activation(vT[vrow:vrow + cw, :], pv[0:cw, :], AF.Copy)
        pu = ps_v.tile([128, N], F32, tag="pvu")
        for b in range(2):
            sl = slice(512 * b, 512 * (b + 1))
            for dc in range(DCC):
                t, lo, hi = srcs[dc]
                nc.tensor.matmul(pu[0:cw, sl], W[("wab", li, dc)][:, csl], t[lo:hi, sl],
                                 start=(dc == 0), stop=(dc == DCC - 1))
        nc.scalar.activation(uT[urow:urow + cw, :], pu[0:cw, :], AF.Copy)

    # ---- normalization state ----
    norm = {0: (ones_r, {0: b0p})}
    statg_of = {}

    def make_stats_coll(li, stat, CC):
        def task():
            bi = dr.tile([128, 2 * CC], F32, tag=f"cc_in{li}", name=f"cc_in{li}")
            bo = dr.tile([128, 2 * CC], F32, tag=f"cc_out{li}", name=f"cc_out{li}")
            nc.scalar.dma_start(bi[:], stat[:])
            nc.gpsimd.collective_compute("AllReduce", ALU.add,
                                         replica_groups=replica,
                                         ins=[bi[:].opt()], outs=[bo[:].opt()])
            statg = sm.tile([128, 2 * CC], F32, tag=f"statg{li}", name=f"statg{li}")
            nc.scalar.dma_start(statg[:], bo[:])
            statg_of[li] = statg
        return task

    def make_fin(li, CC):
        def task():
            statg = statg_of[li]
            mu = sm.tile([128, CC], F32, tag=f"mu{li}", name=f"mu{li}")
            nc.scalar.activation(
                mu[:], statg[:].rearrange("p (c k) -> p c k", k=2)[:, :, 0],
                AF.Copy, scale=1.0 / TOTAL_NODES)
            musq = sm.tile([128, CC], F32, tag=f"musq{li}", name=f"musq{li}")
            nc.scalar.activation(musq[:], mu[:], AF.Square)
            pr = ps_v.tile([1, 1], F32, tag="pvs")
            for cc in range(CC):
                nc.tensor.matmul(pr[:], statg[:, 2 * cc + 1:2 * cc + 2], ones_tn[:],
                                 start=(cc == 0), stop=False)
            for cc in range(CC):
                nc.tensor.matmul(pr[:], musq[:, cc:cc + 1], ones_neg[:],
                                 start=False, stop=(cc == CC - 1))
            rsq = sm.tile([1, 1], F32, tag=f"rsq{li}", name=f"rsq{li}")
            nc.scalar.activation(rsq[:], pr[:], AF.Sqrt, bias=epsT[:])
            rsc = sm.tile([1, 1], F32, tag=f"rsc{li}", name=f"rsc{li}")
            nc.vector.reciprocal(rsc[:], rsq[:])
            prb = ps_v.tile([128, 1], F32, tag="pvs")
            nc.tensor.matmul(prb[:], ones1[:], rsc[:], start=True, stop=True)
            rcol = sm.tile([128, 1], F32, tag=f"rcol{li}", name=f"rcol{li}")
            nc.scalar.activation(rcol[:], prb[:], AF.Copy)
            rneg = sm.tile([128, 1], F32, tag=f"rneg{li}", name=f"rneg{li}")
            nc.scalar.activation(rneg[:], prb[:], AF.Copy, scale=-1.0)
            if li < 2:
                Dn, Cn = LAYERS[li + 1]
                CCn = _ccdiv(Cn)
                DCCn = _ccdiv(Dn)
                bt = sm.tile([128, CCn], F32, tag=f"badj{li + 1}",
                             name=f"badj{li + 1}")
                for ccn in range(CCn):
                    cwn = min(128, Cn - 128 * ccn)
                    csln = slice(128 * ccn, 128 * ccn + cwn)
                    pb = ps_v.tile([128, 1], F32, tag="pvs")
                    for dc in range(DCCn):
                        nc.tensor.matmul(pb[0:cwn, :], W[("wa", li + 1, dc)][:, csln],
                                         mu[0:min(128, Dn - 128 * dc), dc:dc + 1],
                                         start=(dc == 0), stop=(dc == DCCn - 1))
                    nc.scalar.activation(bt[0:cwn, ccn:ccn + 1], pb[0:cwn, :],
                                         AF.Identity, scale=rneg[0:cwn, :],
                                         bias=W[("b", li + 1)][0:cwn, ccn:ccn + 1])
                norm[li + 1] = (rcol, {cc: bt for cc in range(CCn)})
            else:
                norm["head"] = (rcol, mu)
        return task

    def gather_tasks(wrap, vT, a_t, cw):
        """Per-quarter gather (Pool) + slot reduce (DVE) into a_t raw."""
        tasks = []
        for q in range(NQ):
            def tq(q=q):
                qsl = slice(HQ * q, HQ * (q + 1))
                gout = gt.tile([128, KSLOT * HQ], F32, tag="gout", name=f"gout{q}")
                nc.gpsimd.ap_gather(gout[:], vT[:], wrap[:, qsl],
                                    channels=128, num_elems=N, d=1,
                                    num_idxs=KSLOT * HQ)
                nc.vector.tensor_reduce(a_t[0:cw, qsl],
                                        gout[0:cw, :].rearrange(
                                            "p (i s) -> p i s",
                                            s=KSLOT)[:, :, 0:KTOP],
                                        AX.X, ALU.max)
            tasks.append(tq)
        return tasks

    def add_task(a_t, uT, cw):
        def t():
            nc.gpsimd.tensor_tensor(a_t[0:cw, :], a_t[0:cw, :], uT[0:cw, :], ALU.add)
        return t

    def relu_task(a_t, cw, rcol, badj_ap, rs, qs):
        def t():
            nc.scalar.activation(a_t[0:cw, :], a_t[0:cw, :], AF.Relu,
                                 scale=rcol, bias=badj_ap, accum_out=rs)
            sqh = wk.tile([128, N], F32, tag="sqh")
            nc.scalar.activation(sqh[0:cw, :], a_t[0:cw, :], AF.Square,
                                 accum_out=qs)
        return t

    # =================== emission ===================
    for li, (D, C) in enumerate(LAYERS):
        CC = _ccdiv(C)
        rsb = sm.tile([128, 2 * CC, G], F32, tag=f"rsb{li}", name=f"rsb{li}")

        if li == 0:
            for pair in range(2):
                gA, gB = 2 * pair, 2 * pair + 1
                wrap = kp.tile([128, N], I16, tag="wrap", name=f"wrap0_{pair}")
                topk_wrap(0, D, gA, wrap, 0)
                topk_wrap(0, D, gB, wrap, 64)
                drain()
                replicate_wrap(wrap, 0, full=False)
                replicate_wrap(wrap, 64, full=False)
                vT = wk.tile([128, N], F32, tag="vT", name=f"vT0_{pair}")
                uT = wk.tile([128, N], F32, tag="uT", name=f"uT0_{pair}")
                uv_mats(0, D, gA, slice(0, 64), 64, vT, 0, uT, 0)
                uv_mats(0, D, gB, slice(0, 64), 64, vT, 64, uT, 64)
                a_t = feat.tile([128, N], F32, tag="feat", name=f"a1_{pair}")
                rcol0, badj0 = norm[0]
                tasks = gather_tasks(wrap, vT, a_t, 128)
                tasks.append(add_task(a_t, uT, 128))
                tasks.append(relu_task(a_t, 128, rcol0, badj0[0][:, 0:1],
                                       rsb[:, 0, 2 * pair:2 * pair + 1],
                                       rsb[:, 1, 2 * pair:2 * pair + 1]))

                def ab_copy(pair=pair, a_t=a_t, gA=gA, gB=gB):
                    a_b = feat.tile([128, N], F32, tag="feat", name=f"a1b_{pair}")
                    nc.vector.tensor_copy(a_b[0:64, :], a_t[64:128, :])
                    a_cur[(gA, 0)] = (a_t, 0, 64)
                    a_cur[(gB, 0)] = (a_b, 0, 64)
                tasks.append(ab_copy)
                if pair == 1:
                    def l0_stats(rsb=rsb, CC=CC):
                        for p2 in range(2):
                            for k in range(2):
                                nc.vector.tensor_copy(
                                    rsb[0:64, k, 2 * p2 + 1:2 * p2 + 2],
                                    rsb[64:128, k, 2 * p2:2 * p2 + 1])
                        stat = sm.tile([128, 2], F32, tag="stat0", name="stat0")
                        nc.vector.memset(stat[:], 0.0)
                        for k in range(2):
                            nc.vector.tensor_reduce(stat[0:64, k:k + 1],
                                                    rsb[0:64, k, :], AX.X, ALU.add)
                        make_stats_coll(0, stat, CC)()
                    tasks.append(l0_stats)
                pending.extend(tasks)
        else:
            for g in range(G):
                wrap = kp.tile([128, N], I16, tag="wrap", name=f"wrap{li}_{g}")
                topk_wrap(li, D, g, wrap, 0)
                drain()
                replicate_wrap(wrap, 0, full=True)
                vts = []
                for cc in range(CC):
                    cw = min(128, C - 128 * cc)
                    csl = slice(128 * cc, 128 * cc + cw)
                    vT = wk.tile([128, N], F32, tag="vT", name=f"vT{li}_{g}_{cc}")
                    uT = wk.tile([128, N], F32, tag="uT", name=f"uT{li}_{g}_{cc}")
                    uv_mats(li, D, g, csl, cw, vT, 0, uT, 0)
                    vts.append((cc, cw, vT, uT))
                tasks = []
                a_new = []
                for cc, cw, vT, uT in vts:
                    a_t = feat.tile([128, N], F32, tag="feat",
                                    name=f"a{li + 1}_{g}_{cc}")
                    a_new.append((cc, a_t, cw))
                    tasks.extend(gather_tasks(wrap, vT, a_t, cw))
                    tasks.append(add_task(a_t, uT, cw))
                if g == 0:
                    tasks.append(make_fin(li - 1, _ccdiv(LAYERS[li - 1][1])))
                for cc, a_t, cw in a_new:
                    def rl(cc=cc, a_t=a_t, cw=cw, li=li, g=g, rsb=rsb):
                        rcol_p, badj_p = norm[li]
                        relu_task(a_t, cw, rcol_p, badj_p[cc][0:cw, cc:cc + 1],
                                  rsb[0:cw, 2 * cc, g:g + 1],
                                  rsb[0:cw, 2 * cc + 1, g:g + 1])()
                    tasks.append(rl)

                def upd(g=g, a_new=tuple(a_new)):
                    for cc, a_t, cw in a_new:
                        a_cur[(g, cc)] = (a_t, 0, cw)
                upd()
                if g == G - 1:
                    def lstats(li=li, rsb=rsb, CC=CC, C=C):
                        stat = sm.tile([128, 2 * CC], F32, tag=f"stat{li}",
                                       name=f"stat{li}")
                        nc.vector.memset(stat[:], 0.0)
                        for cc in range(CC):
                            cw = min(128, C - 128 * cc)
                            for k in range(2):
                                nc.vector.tensor_reduce(
                                    stat[0:cw, 2 * cc + k:2 * cc + k + 1],
                                    rsb[0:cw, 2 * cc + k, :], AX.X, ALU.add)
                        make_stats_coll(li, stat, CC)()
                    tasks.append(lstats)
                pending.extend(tasks)

    drain()

    # ---- head ----
    gmat = sm.tile([128, 2, G], F32, tag="gmat")
    for g in range(G):
        for cc in range(2):
            t, lo, hi = a_cur[(g, cc)]
            nc.vector.tensor_reduce(gmat[:, cc, g:g + 1], t[:], AX.X, ALU.max)
    make_fin(2, 2)()
    rcol3, mu3 = norm["head"]
    for cc in range(2):
        nc.vector.tensor_scalar(gmat[:, cc, :], gmat[:, cc, :],
                                scalar1=mu3[:, cc:cc + 1], scalar2=rcol3[:],
                                op0=ALU.subtract, op1=ALU.mult)
    ph = ps_v.tile([64, G], F32, tag="pvs")
    for cc in range(2):
        nc.tensor.matmul(ph[:], wl1[:, cc, :], gmat[:, cc, :],
                         start=(cc == 0), stop=(cc == 1))
    hh = sm.tile([64, G], F32, tag="hh")
    nc.scalar.activation(hh[:], ph[:], AF.Relu, bias=bl1[:])
    po = ps_v.tile([2, G], F32, tag="pvs")
    nc.tensor.matmul(po[:], wl2[:], hh[:], start=True, stop=True)
    oo = sm.tile([2, G], F32, tag="oo")
    nc.scalar.activation(oo[:], po[:], AF.Identity, bias=bl2[:])
    nc.sync.dma_start(outs["out"][:], oo[:])
    es.close()


def _host_weights(inputs):
    w = {}
    for li, (D, C) in enumerate(LAYERS):
        Wl = np.asarray(inputs[f"W{li + 1}"], dtype=np.float32)
        w[f"wab{li}"] = np.ascontiguousarray(Wl[:D] - Wl[D:])
        w[f"wb{li}"] = np.ascontiguousarray(Wl[D:])
        w[f"wa{li}"] = np.ascontiguousarray(Wl[:D])
        w[f"b{li}"] = np.ascontiguousarray(
            np.asarray(inputs[f"b{li + 1}"], dtype=np.float32).reshape(C, 1))
    w["wl1"] = np.ascontiguousarray(
        np.asarray(inputs["Wl1"], dtype=np.float32).reshape(2, 128, 64))
    w["bl1"] = np.asarray(inputs["bl1"], dtype=np.float32).reshape(64, 1).copy()
    w["wl2"] = np.ascontiguousarray(np.asarray(inputs["Wl2"], dtype=np.float32))
    w["bl2"] = np.asarray(inputs["bl2"], dtype=np.float32).reshape(2, 1).copy()
    w["eye"] = np.eye(128, dtype=np.float32)
    return w


_CACHED = {}


def _get_module():
    if "nc" in _CACHED:
        return _CACHED["nc"]
    nc = bacc.Bacc("TRN2", target_bir_lowering=False, debug=False, num_devices=N_CORES)
    ins = {"pos": nc.dram_tensor("pos", (G, N, 2), F32, kind="ExternalInput")}
    for li, (D, C) in enumerate(LAYERS):
        ins[f"wab{li}"] = nc.dram_tensor(f"wab{li}", (D, C), F32, kind="ExternalInput")
        ins[f"wb{li}"] = nc.dram_tensor(f"wb{li}", (D, C), F32, kind="ExternalInput")
        ins[f"wa{li}"] = nc.dram_tensor(f"wa{li}", (D, C), F32, kind="ExternalInput")
        ins[f"b{li}"] = nc.dram_tensor(f"b{li}", (C, 1), F32, kind="ExternalInput")
    ins["wl1"] = nc.dram_tensor("wl1", (2, 128, 64), F32, kind="ExternalInput")
    ins["bl1"] = nc.dram_tensor("bl1", (64, 1), F32, kind="ExternalInput")
    ins["wl2"] = nc.dram_tensor("wl2", (64, 2), F32, kind="ExternalInput")
    ins["bl2"] = nc.dram_tensor("bl2", (2, 1), F32, kind="ExternalInput")
    ins["eye"] = nc.dram_tensor("eye", (128, 128), F32, kind="ExternalInput")
    outs = {"out": nc.dram_tensor("out", (2, G), F32, kind="ExternalOutput")}
    with tile.TileContext(nc) as tc:
        _build(tc, nc, ins, outs, n_cores=N_CORES)
    nc.compile()
    _CACHED["nc"] = nc
    return nc


def kernel(**inputs):
    pos = np.ascontiguousarray(np.asarray(inputs["pos"], dtype=np.float32))
    w = _host_weights(inputs)
    nc = _get_module()
    in_maps = []
    for core in range(N_CORES):
        m = {"pos": np.ascontiguousarray(pos[core * G:(core + 1) * G])}
        m.update(w)
        in_maps.append(m)
    res = run_bass_kernel_spmd(nc, in_maps, list(range(N_CORES)))
    outs = [res.results[c]["out"].T for c in range(N_CORES)]  # each [G, 2]
    return np.concatenate(outs, axis=0).astype(np.float32)
